# revision 1
# baseline (speedup 1.0000x reference)
"""APPNP (gnn_message_passing) kernel for 8 axon-tunneled TRN2 NeuronCores.

Self-contained: takes FULL unsharded inputs, shards/preprocesses on host,
compiles and runs a Bass kernel via run_bass_kernel_spmd, returns the FULL
[100000, 16] float32 log-softmax output.
"""
"""APPNP kernel for 8 TRN2 NeuronCores.

Per NC k (dest rows [R*k, R*(k+1)), R = N/8):

Stage A: latent1^T = relu(W1^T @ S^T + b1); z^T = W2^T @ latent1^T + b2,
  with dense S^T slice [F_pad, R_pad] bf16 streamed from HBM (PE matmuls).

Propagation (feature-major [16, nodes]):
  - p tables in SBUF [128, NQ] fp32: group g (16 partitions) holds source
    quarter q=g&3; groups 0-3 serve dest half 0, groups 4-7 dest half 1.
  - Edges bucketed per core g=(h<<2)|q, dest-sorted, chunked by D_CH dests,
    padded to CH slots with a leading dummy slot.
  - Per chunk: ap_gather from table; multiply by 0.9*w; in-place cumsum scan;
    ap_gather extraction of per-dest cumsum ends into `asm` (with zero
    separator columns between chunks).
  - Combine 4 quarter slabs; one global shifted subtract = segment sums;
    fused +0.1*z (z in padded-half layout); DMA p slice out; AllGather;
    strided table reload.

Epilogue: PE transpose to node-major + log_softmax + write y.
"""
from dataclasses import dataclass

import numpy as np
import ml_dtypes

import concourse.bass as bass
import concourse.bacc as bacc
import concourse.mybir as mybir
import concourse.tile as tile
from concourse.masks import make_identity
from concourse.tile_rust import add_dep_helper

F32 = mybir.dt.float32
BF16 = mybir.dt.bfloat16
I16 = mybir.dt.int16
AF = mybir.ActivationFunctionType
ALU = mybir.AluOpType

P = 128


@dataclass
class Cfg:
    N: int = 100000
    F: int = 2000
    HID: int = 64
    LAB: int = 16
    ITERS: int = 10
    ALPHA: float = 0.1
    NCS: int = 8
    D_CH: int = 128           # dests per chunk per core
    CH: int = 0               # slots per chunk (set by prep; data-dependent)
    n_chunks: int = 0         # ceil(HALF / D_CH)
    unroll_iters: bool = True

    @property
    def R(self):
        return self.N // self.NCS

    @property
    def HALF(self):
        return self.R // 2

    @property
    def NQ(self):
        return self.N // 4

    @property
    def F_pad(self):
        return ((self.F + 127) // 128) * 128

    @property
    def R_pad(self):
        return ((self.R + 511) // 512) * 512

    @property
    def HALF_pad(self):       # n_chunks * D_CH
        return self.n_chunks * self.D_CH

    @property
    def ASMW(self):           # col0 zero + per chunk (D_CH ends + 1 zero sep)
        return self.n_chunks * (self.D_CH + 1) + 1


def prep_host(inputs, cfg: Cfg):
    N, NCS = cfg.N, cfg.NCS
    R, HALF, NQ, D_CH = cfg.R, cfg.HALF, cfg.NQ, cfg.D_CH

    feat_rows = np.asarray(inputs["feat_rows"])
    feat_cols = np.asarray(inputs["feat_cols"])
    feat_vals = np.asarray(inputs["feature_values"], dtype=np.float32)
    er = np.asarray(inputs["edge_rows"])
    ec = np.asarray(inputs["edge_cols"])
    ew = np.asarray(inputs["edge_weights"], dtype=np.float32) * (1.0 - cfg.ALPHA)
    W1 = np.asarray(inputs["W1"], dtype=np.float32)
    b1 = np.asarray(inputs["b1"], dtype=np.float32)
    W2 = np.asarray(inputs["W2"], dtype=np.float32)
    b2 = np.asarray(inputs["b2"], dtype=np.float32)

    n_chunks = (HALF + D_CH - 1) // D_CH
    cfg.n_chunks = n_chunks

    nc_of = er // R
    h_of = (er % R) // HALF
    q_of = ec // NQ
    core_of = (h_of << 2) | q_of
    dloc = (er % R) % HALF
    chunk_of = dloc // D_CH

    buckets = {}
    max_edges = 0
    for k in range(NCS):
        m_nc = nc_of == k
        for g in range(8):
            idx = np.nonzero(m_nc & (core_of == g))[0]
            idx = idx[np.argsort(dloc[idx], kind="stable")]
            cb = chunk_of[idx]
            for c in range(n_chunks):
                mm = idx[cb == c]
                buckets[(k, g, c)] = mm
                max_edges = max(max_edges, len(mm))

    CH = ((1 + max_edges + 15) // 16) * 16
    cfg.CH = CH

    # PE combine mask: out row m sums asm partitions of the right half
    hmask = np.zeros((P, 48), np.float32)
    for g in range(8):
        h = g >> 2
        for f in range(16):
            hmask[16 * g + f, 32 * h + f] = 1.0
    in_maps = []
    for k in range(NCS):
        eidx = np.zeros((n_chunks, P, CH // 16), np.int16)
        wstr = np.zeros((n_chunks, P, CH), ml_dtypes.bfloat16)
        xidx = np.zeros((n_chunks, P, D_CH // 16), np.int16)
        for g in range(8):
            q = g & 3
            for c in range(n_chunks):
                e = buckets[(k, g, c)]
                ne = len(e)
                src_loc = np.zeros(CH, np.int16)
                w_loc = np.zeros(CH, np.float32)
                src_loc[1:ne + 1] = (ec[e] - q * NQ).astype(np.int16)
                w_loc[1:ne + 1] = ew[e]
                eidx[c, 16 * g:16 * g + 16, :] = src_loc.reshape(CH // 16, 16).T
                wstr[c, 16 * g:16 * g + 16, :] = w_loc[None, :]
                # ends per dest in the chunk (incl. pad dests at tail)
                dl = dloc[e] - c * D_CH
                cnt = np.bincount(dl, minlength=D_CH)
                ends = np.cumsum(cnt)
                xidx[c, 16 * g:16 * g + 16, :] = (
                    ends.astype(np.int16).reshape(D_CH // 16, 16).T)

        st = np.zeros((cfg.F_pad, cfg.R_pad), np.float32)
        m = (feat_rows >= k * R) & (feat_rows < (k + 1) * R)
        np.add.at(st, (feat_cols[m], feat_rows[m] - k * R), feat_vals[m])
        st = st.astype(ml_dtypes.float8_e4m3)

        w1p = np.zeros((cfg.F_pad, cfg.HID), np.float32)
        w1p[:cfg.F] = W1
        in_maps.append({
            "st": st,
            "w1": w1p.astype(ml_dtypes.float8_e4m3),
            "b1": b1.reshape(cfg.HID, 1).copy(),
            "w2": W2.astype(ml_dtypes.bfloat16),
            "b2": b2.reshape(cfg.LAB, 1).copy(),
            "eidx": eidx,
            "ew": wstr,
            "xidx": xidx,
            "ident": np.tile(np.eye(cfg.LAB, dtype=np.float32), (8, 1)),
            "hmask": hmask,
        })
    return in_maps, {"buckets": buckets}


# ---------------------------------------------------------------------------
def emulate(in_maps, cfg: Cfg):
    """Numpy emulation of the device pipeline (validates host prep)."""
    NCS, R, HALF, NQ = cfg.NCS, cfg.R, cfg.HALF, cfg.NQ
    D_CH, CH, n_chunks = cfg.D_CH, cfg.CH, cfg.n_chunks
    L = cfg.LAB

    zt_all = []
    for k in range(NCS):
        st = in_maps[k]["st"].astype(np.float32)
        lat = np.maximum(
            in_maps[k]["w1"].astype(np.float32).T @ st + in_maps[k]["b1"], 0.0)
        lat = lat.astype(ml_dtypes.bfloat16).astype(np.float32)
        zt = in_maps[k]["w2"].astype(np.float32).T @ lat + in_maps[k]["b2"]
        zt_all.append(zt[:, :R])
    z = np.concatenate(zt_all, axis=1)  # [16, N]

    p = z.copy()
    for _ in range(cfg.ITERS):
        newp = np.zeros_like(p)
        for k in range(NCS):
            pd = np.zeros((L, R), np.float32)
            for g in range(8):
                q, h = g & 3, g >> 2
                tblq = p[:, q * NQ:(q + 1) * NQ]
                for c in range(n_chunks):
                    idx = in_maps[k]["eidx"][c, 16 * g:16 * g + 16].T.reshape(-1)
                    w = in_maps[k]["ew"][c, 16 * g]
                    gath = tblq[:, idx] * w[None, :]
                    cum = np.cumsum(gath, axis=1)
                    ends = in_maps[k]["xidx"][c, 16 * g:16 * g + 16].T.reshape(-1)
                    ext = cum[:, ends]
                    seg = np.empty_like(ext)
                    seg[:, 0] = ext[:, 0]
                    seg[:, 1:] = ext[:, 1:] - ext[:, :-1]
                    lo = h * HALF + c * D_CH
                    hi = min(lo + D_CH, (h + 1) * HALF)
                    pd[:, lo:hi] += seg[:, :hi - lo]
            newp[:, k * R:(k + 1) * R] = pd + cfg.ALPHA * z[:, k * R:(k + 1) * R]
        p = newp
    x = p.T
    m = x.max(1, keepdims=True)
    e = np.exp(x - m)
    return (x - m) - np.log(e.sum(1, keepdims=True))


# ---------------------------------------------------------------------------
def build_kernel(cfg: Cfg):
    NCS, R, HALF, NQ = cfg.NCS, cfg.R, cfg.HALF, cfg.NQ
    D_CH, CH, n_chunks = cfg.D_CH, cfg.CH, cfg.n_chunks
    HID, LAB, F_pad, R_pad = cfg.HID, cfg.LAB, cfg.F_pad, cfg.R_pad
    HP = cfg.HALF_pad
    KT = F_pad // 128
    NT = R_pad // 512
    FP = ((2 * HP + 511) // 512) * 512
    DW = D_CH + 1

    nc = bacc.Bacc("TRN2", target_bir_lowering=False, debug=False,
                   num_devices=NCS)

    F8 = mybir.dt.float8e4
    st_e = nc.declare_dram_parameter("st", [F_pad, R_pad], F8, isOutput=False)
    w1_e = nc.declare_dram_parameter("w1", [F_pad, HID], F8, isOutput=False)
    b1_e = nc.declare_dram_parameter("b1", [HID, 1], F32, isOutput=False)
    w2_e = nc.declare_dram_parameter("w2", [HID, LAB], BF16, isOutput=False)
    b2_e = nc.declare_dram_parameter("b2", [LAB, 1], F32, isOutput=False)
    eidx_e = nc.declare_dram_parameter("eidx", [n_chunks, P, CH // 16], I16,
                                       isOutput=False)
    ew_e = nc.declare_dram_parameter("ew", [n_chunks, P, CH], BF16,
                                     isOutput=False)
    xidx_e = nc.declare_dram_parameter("xidx", [n_chunks, P, D_CH // 16], I16,
                                       isOutput=False)
    ident_e = nc.declare_dram_parameter("ident", [P, LAB], F32,
                                        isOutput=False)
    hmask_e = nc.declare_dram_parameter("hmask", [P, 48], F32, isOutput=False)
    y_e = nc.declare_dram_parameter("y", [FP, LAB], F32, isOutput=True)

    p_slice = nc.dram_tensor("p_slice", [LAB, R], F32)
    gathered = nc.dram_tensor("gathered", [NCS * LAB, R], F32,
                              addr_space="Shared")

    with tile.TileContext(nc) as tc:
        _frees = []

        def talloc(shape, dtype, name):
            t, _f = tc.tile(shape, dtype, name=name)
            _frees.append(_f)
            return t

        with (
            tc.tile_pool(name="pch", bufs=2) as pch,
            tc.tile_pool(name="ps", bufs=2, space="PSUM") as ps,
        ):
            # pd/zth: rows 0:16 = half0, rows 32:48 = half1 (psum alignment)
            pdt = talloc([P, HP], F32, "pdt")
            nc.vector.memset(pdt[:], 0.0)
            zth = talloc([P, HP], F32, "zth")
            nc.vector.memset(zth[:], 0.0)

            F8 = mybir.dt.float8e4
            w1_sb = talloc([P, KT, HID], F8, "w1_sb")
            nc.sync.dma_start(out=w1_sb[:], in_=w1_e[:].rearrange(
                "(kt p) h -> p kt h", p=P))
            b1_sb = talloc([HID, 1], F32, "b1_sb")
            nc.sync.dma_start(out=b1_sb[:], in_=b1_e[:])
            w2_sb = talloc([HID, LAB], BF16, "w2_sb")
            nc.sync.dma_start(out=w2_sb[:], in_=w2_e[:])
            b2_sb = talloc([LAB, 1], F32, "b2_sb")
            nc.sync.dma_start(out=b2_sb[:], in_=b2_e[:])
            ident = talloc([P, LAB], F32, "ident")
            nc.sync.dma_start(out=ident[:], in_=ident_e[:])
            hmask = talloc([P, 48], F32, "hmask")
            nc.sync.dma_start(out=hmask[:], in_=hmask_e[:])
            ones = talloc([P, 1], F32, "ones")
            nc.vector.memset(ones[:], 1.0)

            # ---------------- stage A ----------------
            with tc.tile_pool(name="sarhs", bufs=2) as sarhs:
                for nt in range(NT):
                    rhs = sarhs.tile([P, KT, 512], F8, name="rhs")
                    nc.sync.dma_start(
                        out=rhs[:],
                        in_=st_e[:, nt * 512:(nt + 1) * 512].rearrange(
                            "(kt p) n -> p kt n", p=P))
                    ps1 = ps.tile([HID, 512], F32, name="ps1", space="PSUM")
                    for kt in range(KT):
                        nc.tensor.matmul(
                            out=ps1[:], lhsT=w1_sb[:, kt, :], rhs=rhs[:, kt, :],
                            start=(kt == 0), stop=(kt == KT - 1))
                    lat = sarhs.tile([HID, 512], BF16, name="lat")
                    nc.scalar.activation(out=lat[:], in_=ps1[:], func=AF.Relu,
                                         bias=b1_sb[:, 0:1])
                    ps2 = ps.tile([LAB, 512], F32, name="ps2", space="PSUM")
                    nc.tensor.matmul(out=ps2[:], lhsT=w2_sb[:], rhs=lat[:],
                                     start=True, stop=True)
                    zchunk = sarhs.tile([LAB, 512], F32, name="zchunk")
                    nc.vector.tensor_scalar_add(
                        out=zchunk[:], in0=ps2[:], scalar1=b2_sb[:, 0:1])
                    n0 = nt * 512
                    n1 = min(n0 + 512, R)
                    if n0 < R:
                        nc.sync.dma_start(out=p_slice[:, n0:n1],
                                          in_=zchunk[:, 0:n1 - n0])
                        a1 = min(n1, HALF)
                        if n0 < a1:
                            nc.vector.tensor_copy(
                                out=zth[0:16, n0:a1],
                                in_=zchunk[:, 0:a1 - n0])
                        b0 = max(n0, HALF)
                        if b0 < n1:
                            # cross-partition (rows 32:48) -> SBUF DMA
                            nc.sync.dma_start(
                                out=zth[32:48, b0 - HALF:n1 - HALF],
                                in_=zchunk[:, b0 - n0:n1 - n0])

            # ---------------- propagation state ----------------
            table = talloc([P, NQ], F32, "table")
            nc.vector.memset(table[:], 0.0)
            eidx_sb = talloc([P, n_chunks * (CH // 16)], I16, "eidx_sb")
            xidx_sb = talloc([P, n_chunks * (D_CH // 16)], I16, "xidx_sb")
            idx_loads = []
            for c in range(n_chunks):
                idx_loads.append(nc.sync.dma_start(
                    out=eidx_sb[:, c * (CH // 16):(c + 1) * (CH // 16)],
                    in_=eidx_e[c]))
                idx_loads.append(nc.sync.dma_start(
                    out=xidx_sb[:, c * (D_CH // 16):(c + 1) * (D_CH // 16)],
                    in_=xidx_e[c]))
            asm_w = [talloc([P, DW], F32, f"asmw{i}") for i in range(2)]
            for t in asm_w:
                nc.vector.memset(t[:], 0.0)
            hw_t = [talloc([P, DW], F32, f"hw{i}") for i in range(2)]

            def dep(a, b, sync=True):
                add_dep_helper(a.ins, b.ins, sync=sync, reason="manual")

            state = {"last_pool": None, "reloads": [], "idx_loads": idx_loads,
                     "scan_ring": [None, None, None], "mm_ring": [None, None],
                     "last_gather": None, "pd_readers": []}

            def pool_chain(inst):
                if state["last_pool"] is not None:
                    dep(inst, state["last_pool"], sync=False)
                state["last_pool"] = inst

            def reload_tables():
                nc.gpsimd.collective_compute(
                    "AllGather", ALU.bypass,
                    replica_groups=[list(range(NCS))],
                    ins=[p_slice[:]], outs=[gathered[:]])
                state["reloads"] = []
                for a in range(2):
                    for hh in range(2):
                        src = bass.AP(
                            tensor=gathered,
                            offset=16 * R * hh,
                            ap=[[32 * R, 4], [R, 16], [1, R]])
                        ld = nc.sync.dma_start(
                            out=table[a * 64:(a + 1) * 64,
                                      hh * R:(hh + 1) * R], in_=src)
                        if state["last_gather"] is not None:
                            dep(ld, state["last_gather"])
                        state["reloads"].append(ld)

            reload_tables()  # p0 = z (p_slice written during stage A)

            def iteration(last: bool):
                LA = 3  # lookahead: gathers run ahead of extraction phase
                scans = {}

                def emit_gather(c):
                    g_out = pch.tile([P, CH], F32, name=f"g_out{c % LA}",
                                     bufs=1)
                    gather = nc.gpsimd.ap_gather(
                        out_ap=g_out[:].rearrange("p (n o) -> p n o", o=1),
                        in_ap=table[:].rearrange("p (n o) -> p n o", o=1),
                        idxs_ap=eidx_sb[:, c * (CH // 16):(c + 1) * (CH // 16)],
                        channels=P, num_elems=NQ, d=1, num_idxs=CH)
                    pool_chain(gather)
                    for ld in state["reloads"]:
                        dep(gather, ld)
                    if c == 0:
                        for ld in state["idx_loads"]:
                            dep(gather, ld)
                        state["idx_loads"] = []
                        for rd in state["pd_readers"]:
                            dep(gather, rd)
                        state["pd_readers"] = []
                    if state["scan_ring"][c % LA] is not None:
                        dep(gather, state["scan_ring"][c % LA])
                    state["last_gather"] = gather
                    wch = pch.tile([P, CH], BF16, name=f"wch{c % LA}", bufs=1)
                    nc.sync.dma_start(out=wch[:], in_=ew_e[c])
                    mult = nc.vector.tensor_tensor(out=g_out[:], in0=g_out[:],
                                                   in1=wch[:], op=ALU.mult)
                    dep(mult, gather)
                    scan = nc.vector.tensor_tensor_scan(
                        out=g_out[:], data0=ones[:].to_broadcast([P, CH]),
                        data1=g_out[:], initial=0.0,
                        op0=ALU.mult, op1=ALU.add)
                    state["scan_ring"][c % LA] = scan
                    scans[c] = (scan, g_out)

                def emit_tail(c):
                    scan, g_out = scans.pop(c)
                    aw = asm_w[c % 2]
                    ext = nc.gpsimd.ap_gather(
                        out_ap=aw[:, 1:DW].rearrange("p (n o) -> p n o", o=1),
                        in_ap=g_out[:].rearrange("p (n o) -> p n o", o=1),
                        idxs_ap=xidx_sb[:, c * (D_CH // 16):
                                        (c + 1) * (D_CH // 16)],
                        channels=P, num_elems=CH, d=1, num_idxs=D_CH)
                    pool_chain(ext)
                    dep(ext, scan)
                    if state["mm_ring"][c % 2] is not None:
                        dep(ext, state["mm_ring"][c % 2])
                    psc = ps.tile([48, DW], F32, name="psc", space="PSUM")
                    mm = nc.tensor.matmul(out=psc[:], lhsT=hmask[:],
                                          rhs=aw[:], start=True, stop=True)
                    dep(mm, ext)
                    state["mm_ring"][c % 2] = mm
                    hw = hw_t[c % 2]
                    nc.vector.tensor_copy(out=hw[0:48, :], in_=psc[:])
                    for h in range(2):
                        nc.vector.tensor_tensor(
                            out=pdt[32 * h:32 * h + 16,
                                    c * D_CH:(c + 1) * D_CH],
                            in0=hw[32 * h:32 * h + 16, 1:DW],
                            in1=hw[32 * h:32 * h + 16, 0:DW - 1],
                            op=ALU.subtract)

                for s_ in range(n_chunks + LA):
                    if s_ >= LA:
                        emit_tail(s_ - LA)
                    if s_ < n_chunks:
                        emit_gather(s_)
                # alpha*z add over the whole region
                nc.vector.scalar_tensor_tensor(
                    out=pdt[0:48, :], in0=zth[0:48, :], scalar=cfg.ALPHA,
                    in1=pdt[0:48, :], op0=ALU.mult, op1=ALU.add)
                if not last:
                    d0 = nc.sync.dma_start(out=p_slice[:, 0:HALF],
                                           in_=pdt[0:16, 0:HALF])
                    d1 = nc.sync.dma_start(out=p_slice[:, HALF:R],
                                           in_=pdt[32:48, 0:HALF])
                    state["pd_readers"] = [d0, d1]
                    reload_tables()

            if cfg.unroll_iters:
                for it in range(cfg.ITERS):
                    iteration(last=(it == cfg.ITERS - 1))
            else:
                with tc.For_i(0, cfg.ITERS - 1, 1) as _i:
                    iteration(last=False)
                iteration(last=True)

            # ------------- epilogue: transpose + log_softmax -------------
            total_chunks = 2 * n_chunks
            b = 0
            done = 0
            while done < total_chunks:
                nchk = min(4, total_chunks - done)
                ps3 = ps.tile([P, 4 * LAB], F32, name="ps3", space="PSUM")
                for t in range(nchk):
                    gc = done + t
                    h, c = gc // n_chunks, gc % n_chunks
                    nc.tensor.transpose(
                        out=ps3[:, t * LAB:(t + 1) * LAB],
                        in_=pdt[32 * h:32 * h + 16,
                                c * D_CH:(c + 1) * D_CH],
                        identity=ident[32 * h:32 * h + 16, :])
                sb = pch.tile([P, 4, LAB], F32, name="sm_sb")
                nc.vector.tensor_copy(
                    out=sb[:, 0:nchk, :].rearrange("p a l -> p (a l)"),
                    in_=ps3[:, 0:nchk * LAB])
                mx = pch.tile([P, 4, 1], F32, name="sm_mx")
                nc.vector.tensor_reduce(out=mx[:, 0:nchk], in_=sb[:, 0:nchk],
                                        axis=mybir.AxisListType.X, op=ALU.max)
                nc.vector.tensor_tensor(
                    out=sb[:, 0:nchk], in0=sb[:, 0:nchk],
                    in1=mx[:, 0:nchk].to_broadcast([P, nchk, LAB]),
                    op=ALU.subtract)
                ex = pch.tile([P, 4, LAB], F32, name="sm_ex")
                nc.scalar.activation(out=ex[:, 0:nchk], in_=sb[:, 0:nchk],
                                     func=AF.Exp)
                sm = pch.tile([P, 4, 1], F32, name="sm_sm")
                nc.vector.tensor_reduce(out=sm[:, 0:nchk], in_=ex[:, 0:nchk],
                                        axis=mybir.AxisListType.X, op=ALU.add)
                lg = pch.tile([P, 4, 1], F32, name="sm_lg")
                nc.scalar.activation(out=lg[:, 0:nchk], in_=sm[:, 0:nchk],
                                     func=AF.Ln)
                nc.vector.tensor_tensor(
                    out=sb[:, 0:nchk], in0=sb[:, 0:nchk],
                    in1=lg[:, 0:nchk].to_broadcast([P, nchk, LAB]),
                    op=ALU.subtract)
                nc.sync.dma_start(
                    out=y_e[:].rearrange("(x p) l -> p x l", p=P)[
                        :, 4 * b:4 * b + nchk, :],
                    in_=sb[:, 0:nchk, :])
                done += nchk
                b += 1
            for _f in reversed(_frees):
                _f()
    nc.compile()
    return nc


def unpack_output(results, cfg: Cfg):
    HP = cfg.HALF_pad
    out = np.zeros((cfg.N, cfg.LAB), np.float32)
    for k in range(cfg.NCS):
        y = results[k]["y"]
        out[k * cfg.R:k * cfg.R + cfg.HALF] = y[0:cfg.HALF]
        out[k * cfg.R + cfg.HALF:(k + 1) * cfg.R] = y[HP:HP + cfg.HALF]
    return out


# ---------------------------------------------------------------------------
_CACHE = {}


def kernel(**inputs):
    import numpy as np
    from concourse.bass_utils import run_bass_kernel_spmd

    cfg = Cfg()
    in_maps, _meta = prep_host(inputs, cfg)
    key = (cfg.CH, cfg.n_chunks)
    if key not in _CACHE:
        _CACHE[key] = build_kernel(cfg)
    nc = _CACHE[key]
    r = run_bass_kernel_spmd(nc, in_maps, list(range(cfg.NCS)))
    return unpack_output(r.results, cfg)



# revision 8
# speedup vs baseline: 2.1055x; 2.1055x over previous
"""APPNP (gnn_message_passing) kernel for 8 axon-tunneled TRN2 NeuronCores.

Self-contained: takes FULL unsharded inputs, shards/preprocesses on host,
compiles and runs a Bass kernel via run_bass_kernel_spmd, returns the FULL
[100000, 16] float32 log-softmax output.

v2 design. Per NC k (dest rows [R*k, R*(k+1)), R = N/8 = 12500):

Stage A: latent1^T = relu(W1^T @ S^T + b1); z^T = W2^T @ latent1^T + b2,
  with dense S^T slice [F_pad, R_pad] fp8 streamed from HBM (PE matmuls).
  z^T written to z_dram [16, R] and p_slice [16, R].

Propagation (feature-major [16, nodes]):
  - p table in SBUF [128, NE=12500] fp32: group g (16 partitions) holds
    source EIGHTH g (= core g's node range) — identical layout to the
    AllGather output, so the reload is one contiguous DMA.
  - Edges bucketed by (core k, group g = src//NE, sub c = dloc//512),
    dest-sorted; every bucket padded to CH_SUB slots with slot 0 a dummy.
  - 5 subs form one gather chunk: per chunk, one ap_gather of 5*CH_SUB
    idxs from the table; multiply by 0.9*w (bf16 from HBM); per sub an
    in-place cumsum scan, an ap_gather extracting per-dest cumsum ends
    into aw [P, 512]; two accumulating PE matmuls (hmask, -hmask shifted)
    produce per-dest segment sums in PSUM; one DVE op adds alpha*z and
    lands the [16, 512] result; DMA to p_slice.
  - AllGather p_slice -> gathered [128, R]; contiguous DMA -> table.

Epilogue: read p_slice back per 512 cols, PE transpose to node-major,
log_softmax, write y [R_pad_y, 16].
"""
from dataclasses import dataclass

import numpy as np
import ml_dtypes

import concourse.bass as bass
import concourse.bacc as bacc
import concourse.mybir as mybir
import concourse.tile as tile
from concourse.tile_rust import add_dep_helper

F32 = mybir.dt.float32
BF16 = mybir.dt.bfloat16
I16 = mybir.dt.int16
AF = mybir.ActivationFunctionType
ALU = mybir.AluOpType

P = 128


@dataclass
class Cfg:
    N: int = 100000
    F: int = 2000
    HID: int = 64
    LAB: int = 16
    ITERS: int = 10
    ALPHA: float = 0.1
    NCS: int = 8
    D_SUB: int = 512          # dests per sub-chunk (PSUM bank: <=512 fp32)
    CH_SUB: int = 0           # slots per (group, sub) bucket (data-dep)
    SPG: int = 5              # subs per gather chunk
    use_collective: bool = True

    @property
    def R(self):
        return self.N // self.NCS

    @property
    def NE(self):             # sources per group (eighth)
        return self.N // 8

    @property
    def n_subs(self):
        return (self.R + self.D_SUB - 1) // self.D_SUB

    @property
    def n_gch(self):
        return (self.n_subs + self.SPG - 1) // self.SPG

    @property
    def F_pad(self):
        return ((self.F + 127) // 128) * 128

    @property
    def R_pad(self):
        return ((self.R + 511) // 512) * 512


def prep_host(inputs, cfg: Cfg):
    N, NCS, R, NE, D_SUB = cfg.N, cfg.NCS, cfg.R, cfg.NE, cfg.D_SUB
    n_subs = cfg.n_subs

    feat_rows = np.asarray(inputs["feat_rows"])
    feat_cols = np.asarray(inputs["feat_cols"])
    feat_vals = np.asarray(inputs["feature_values"], dtype=np.float32)
    er = np.asarray(inputs["edge_rows"])
    ec = np.asarray(inputs["edge_cols"])
    ew = np.asarray(inputs["edge_weights"], dtype=np.float32) * (1.0 - cfg.ALPHA)
    W1 = np.asarray(inputs["W1"], dtype=np.float32)
    b1 = np.asarray(inputs["b1"], dtype=np.float32)
    W2 = np.asarray(inputs["W2"], dtype=np.float32)
    b2 = np.asarray(inputs["b2"], dtype=np.float32)

    nc_of = er // R
    dloc = er % R
    g_of = ec // NE
    sub_of = dloc // D_SUB
    src_loc = (ec - g_of * NE).astype(np.int16)

    # order edges by (core, group, sub, dloc)
    order = np.lexsort((dloc, sub_of, g_of, nc_of))
    key = ((nc_of * 8 + g_of) * n_subs + sub_of)
    cnt = np.bincount(key, minlength=NCS * 8 * n_subs)
    CH_SUB = ((1 + int(cnt.max()) + 15) // 16) * 16
    cfg.CH_SUB = CH_SUB
    starts = np.zeros(NCS * 8 * n_subs + 1, np.int64)
    np.cumsum(cnt, out=starts[1:])

    # slot position of each (sorted) edge: bucket_base + 1 + rank_in_bucket
    ks = key[order]
    rank = np.arange(len(order)) - starts[ks]
    GCH = cfg.SPG * CH_SUB

    hmask = np.zeros((P, 16), np.float32)
    hmaskn = np.zeros((P, 16), np.float32)
    for g in range(8):
        for f in range(16):
            hmask[16 * g + f, f] = 1.0
            hmaskn[16 * g + f, f] = -1.0

    cnt_r = cnt.reshape(NCS, 8, n_subs)
    starts_r = starts[:-1].reshape(NCS, 8, n_subs)
    sorted_src = src_loc[order]
    sorted_w = ew[order]
    sorted_dloc = dloc[order]

    in_maps = []
    for k in range(NCS):
        eidx = np.zeros((cfg.n_gch, P, GCH // 16), np.int16)
        wstr = np.zeros((cfg.n_gch, P, GCH), ml_dtypes.bfloat16)
        xidx = np.zeros((n_subs, P, D_SUB // 16), np.int16)
        for g in range(8):
            for c in range(n_subs):
                ne = cnt_r[k, g, c]
                s0 = starts_r[k, g, c]
                gc, sl = c // cfg.SPG, c % cfg.SPG
                src_b = np.zeros(CH_SUB, np.int16)
                w_b = np.zeros(CH_SUB, np.float32)
                src_b[1:ne + 1] = sorted_src[s0:s0 + ne]
                w_b[1:ne + 1] = sorted_w[s0:s0 + ne]
                off = sl * CH_SUB
                eidx[gc, 16 * g:16 * g + 16, off // 16:(off + CH_SUB) // 16] = (
                    src_b.reshape(CH_SUB // 16, 16).T)
                wstr[gc, 16 * g:16 * g + 16, off:off + CH_SUB] = w_b[None, :]
                dl = sorted_dloc[s0:s0 + ne] - c * D_SUB
                bc = np.bincount(dl, minlength=D_SUB)
                ends = np.cumsum(bc)[:D_SUB]
                xidx[c, 16 * g:16 * g + 16, :] = (
                    ends.astype(np.int16).reshape(D_SUB // 16, 16).T)

        st = np.zeros((cfg.F_pad, cfg.R_pad), np.float32)
        m = (feat_rows >= k * R) & (feat_rows < (k + 1) * R)
        np.add.at(st, (feat_cols[m], feat_rows[m] - k * R), feat_vals[m])
        st = st.astype(ml_dtypes.float8_e4m3)

        w1p = np.zeros((cfg.F_pad, cfg.HID), np.float32)
        w1p[:cfg.F] = W1
        in_maps.append({
            "st": st,
            "w1": w1p.astype(ml_dtypes.float8_e4m3),
            "b1": b1.reshape(cfg.HID, 1).copy(),
            "w2": W2.astype(ml_dtypes.bfloat16),
            "b2": b2.reshape(cfg.LAB, 1).copy(),
            "eidx": eidx,
            "ew": wstr,
            "xidx": xidx,
            "ident": np.tile(np.eye(cfg.LAB, dtype=np.float32), (8, 1)),
            "hmask": hmask,
            "hmaskn": hmaskn,
        })
    return in_maps, {}


# ---------------------------------------------------------------------------
def emulate(in_maps, cfg: Cfg):
    """Numpy emulation of the device pipeline (validates host prep)."""
    NCS, R, NE = cfg.NCS, cfg.R, cfg.NE
    D_SUB, CH_SUB, n_subs = cfg.D_SUB, cfg.CH_SUB, cfg.n_subs
    L = cfg.LAB

    zt_all = []
    for k in range(NCS):
        st = in_maps[k]["st"].astype(np.float32)
        lat = np.maximum(
            in_maps[k]["w1"].astype(np.float32).T @ st + in_maps[k]["b1"], 0.0)
        lat = lat.astype(ml_dtypes.bfloat16).astype(np.float32)
        zt = in_maps[k]["w2"].astype(np.float32).T @ lat + in_maps[k]["b2"]
        zt_all.append(zt[:, :R])
    z = np.concatenate(zt_all, axis=1)  # [16, N]

    p = z.copy()
    for _ in range(cfg.ITERS):
        newp = np.zeros_like(p)
        for k in range(NCS):
            pd = np.zeros((L, R), np.float32)
            for c in range(n_subs):
                gc, sl = c // cfg.SPG, c % cfg.SPG
                lo, hi = c * D_SUB, min((c + 1) * D_SUB, R)
                seg_sum = np.zeros((L, D_SUB), np.float32)
                for g in range(8):
                    tbl = p[:, g * NE:(g + 1) * NE]
                    idx = in_maps[k]["eidx"][
                        gc, 16 * g:16 * g + 16,
                        sl * CH_SUB // 16:(sl + 1) * CH_SUB // 16
                    ].T.reshape(-1)
                    w = in_maps[k]["ew"][gc, 16 * g,
                                         sl * CH_SUB:(sl + 1) * CH_SUB]
                    gath = tbl[:, idx] * np.asarray(w, np.float32)[None, :]
                    cum = np.cumsum(gath, axis=1)
                    ends = in_maps[k]["xidx"][
                        c, 16 * g:16 * g + 16].T.reshape(-1)
                    aw = cum[:, ends]
                    seg = np.empty_like(aw)
                    seg[:, 0] = aw[:, 0]
                    seg[:, 1:] = aw[:, 1:] - aw[:, :-1]
                    seg_sum += seg
                pd[:, lo:hi] += seg_sum[:, :hi - lo]
            newp[:, k * R:(k + 1) * R] = pd + cfg.ALPHA * z[:, k * R:(k + 1) * R]
        p = newp
    x = p.T
    m = x.max(1, keepdims=True)
    e = np.exp(x - m)
    return (x - m) - np.log(e.sum(1, keepdims=True))


# ---------------------------------------------------------------------------
def build_kernel(cfg: Cfg):
    NCS, R, NE = cfg.NCS, cfg.R, cfg.NE
    D_SUB, CH_SUB, n_subs = cfg.D_SUB, cfg.CH_SUB, cfg.n_subs
    SPG, n_gch = cfg.SPG, cfg.n_gch
    HID, LAB, F_pad, R_pad = cfg.HID, cfg.LAB, cfg.F_pad, cfg.R_pad
    KT = F_pad // 128
    NT = R_pad // 512
    GCH = SPG * CH_SUB
    FP = ((R + 511) // 512) * 512   # y rows padding (>= R, mult of 512)

    nc = bacc.Bacc("TRN2", target_bir_lowering=False, debug=False,
                   num_devices=NCS)

    F8 = mybir.dt.float8e4
    st_e = nc.declare_dram_parameter("st", [F_pad, R_pad], F8, isOutput=False)
    w1_e = nc.declare_dram_parameter("w1", [F_pad, HID], F8, isOutput=False)
    b1_e = nc.declare_dram_parameter("b1", [HID, 1], F32, isOutput=False)
    w2_e = nc.declare_dram_parameter("w2", [HID, LAB], BF16, isOutput=False)
    b2_e = nc.declare_dram_parameter("b2", [LAB, 1], F32, isOutput=False)
    eidx_e = nc.declare_dram_parameter("eidx", [n_gch, P, GCH // 16], I16,
                                       isOutput=False)
    ew_e = nc.declare_dram_parameter("ew", [n_gch, P, GCH], BF16,
                                     isOutput=False)
    xidx_e = nc.declare_dram_parameter("xidx", [n_subs, P, D_SUB // 16], I16,
                                       isOutput=False)
    ident_e = nc.declare_dram_parameter("ident", [P, LAB], F32,
                                        isOutput=False)
    hmask_e = nc.declare_dram_parameter("hmask", [P, LAB], F32, isOutput=False)
    hmaskn_e = nc.declare_dram_parameter("hmaskn", [P, LAB], F32,
                                         isOutput=False)
    y_e = nc.declare_dram_parameter("y", [FP, LAB], F32, isOutput=True)

    p_slice = nc.dram_tensor("p_slice", [LAB, R], F32)
    z_dram = nc.dram_tensor("z_dram", [LAB, R], F32)
    gathered = nc.dram_tensor("gathered", [NCS * LAB, R], F32,
                              addr_space="Shared")

    with tile.TileContext(nc) as tc:
        _frees = []

        def talloc(shape, dtype, name):
            t, _f = tc.tile(shape, dtype, name=name)
            _frees.append(_f)
            return t

        with (
            tc.tile_pool(name="pch", bufs=2) as pch,
            tc.tile_pool(name="ps", bufs=2, space="PSUM") as ps,
        ):
            w1_sb = talloc([P, KT, HID], F8, "w1_sb")
            nc.sync.dma_start(out=w1_sb[:], in_=w1_e[:].rearrange(
                "(kt p) h -> p kt h", p=P))
            b1_sb = talloc([HID, 1], F32, "b1_sb")
            nc.sync.dma_start(out=b1_sb[:], in_=b1_e[:])
            w2_sb = talloc([HID, LAB], BF16, "w2_sb")
            nc.sync.dma_start(out=w2_sb[:], in_=w2_e[:])
            b2_sb = talloc([LAB, 1], F32, "b2_sb")
            nc.sync.dma_start(out=b2_sb[:], in_=b2_e[:])
            ident = talloc([P, LAB], F32, "ident")
            nc.sync.dma_start(out=ident[:], in_=ident_e[:])
            hmask = talloc([P, LAB], F32, "hmask")
            nc.sync.dma_start(out=hmask[:], in_=hmask_e[:])
            hmaskn = talloc([P, LAB], F32, "hmaskn")
            nc.sync.dma_start(out=hmaskn[:], in_=hmaskn_e[:])
            ones = talloc([P, 1], F32, "ones")
            nc.vector.memset(ones[:], 1.0)

            # ---------------- stage A ----------------
            with tc.tile_pool(name="sarhs", bufs=2) as sarhs:
                for nt in range(NT):
                    rhs = sarhs.tile([P, KT, 512], F8, name="rhs")
                    nc.sync.dma_start(
                        out=rhs[:],
                        in_=st_e[:, nt * 512:(nt + 1) * 512].rearrange(
                            "(kt p) n -> p kt n", p=P))
                    ps1 = ps.tile([HID, 512], F32, name="ps1", space="PSUM")
                    for kt in range(KT):
                        nc.tensor.matmul(
                            out=ps1[:], lhsT=w1_sb[:, kt, :], rhs=rhs[:, kt, :],
                            start=(kt == 0), stop=(kt == KT - 1))
                    lat = sarhs.tile([HID, 512], BF16, name="lat")
                    nc.scalar.activation(out=lat[:], in_=ps1[:], func=AF.Relu,
                                         bias=b1_sb[:, 0:1])
                    ps2 = ps.tile([LAB, 512], F32, name="ps2", space="PSUM")
                    nc.tensor.matmul(out=ps2[:], lhsT=w2_sb[:], rhs=lat[:],
                                     start=True, stop=True)
                    zchunk = sarhs.tile([LAB, 512], F32, name="zchunk")
                    nc.vector.tensor_scalar_add(
                        out=zchunk[:], in0=ps2[:], scalar1=b2_sb[:, 0:1])
                    n0 = nt * 512
                    n1 = min(n0 + 512, R)
                    if n0 < R:
                        nc.sync.dma_start(out=p_slice[:, n0:n1],
                                          in_=zchunk[:, 0:n1 - n0])
                        nc.sync.dma_start(out=z_dram[:, n0:n1],
                                          in_=zchunk[:, 0:n1 - n0])

            # ---------------- propagation state ----------------
            table = talloc([P, NE], F32, "table")
            eidx_sb = talloc([P, n_gch * (GCH // 16)], I16, "eidx_sb")
            xidx_sb = talloc([P, n_subs * (D_SUB // 16)], I16, "xidx_sb")
            idx_loads = []
            for c in range(n_gch):
                idx_loads.append(nc.sync.dma_start(
                    out=eidx_sb[:, c * (GCH // 16):(c + 1) * (GCH // 16)],
                    in_=eidx_e[c]))
            for c in range(n_subs):
                idx_loads.append(nc.sync.dma_start(
                    out=xidx_sb[:, c * (D_SUB // 16):(c + 1) * (D_SUB // 16)],
                    in_=xidx_e[c]))
            aw_t = [talloc([P, D_SUB], F32, f"aw{i}") for i in range(2)]
            wch = talloc([P, GCH], BF16, "wch")

            def dep(a, b, sync=True):
                add_dep_helper(a.ins, b.ins, sync=sync, reason="manual")

            state = {"last_pool": None, "reload": None, "idx_loads": idx_loads,
                     "gout_ring": [None, None], "aw_ring": [None, None],
                     "wch_last_reader": None, "wch_load": None,
                     "last_gather": None, "ps_writers": []}

            def pool_chain(inst):
                if state["last_pool"] is not None:
                    dep(inst, state["last_pool"], sync=False)
                state["last_pool"] = inst

            def reload_tables():
                if cfg.use_collective:
                    cc = nc.gpsimd.collective_compute(
                        "AllGather", ALU.bypass,
                        replica_groups=[list(range(NCS))],
                        ins=[p_slice[:]], outs=[gathered[:]])
                ld = nc.sync.dma_start(out=table[:], in_=gathered[:])
                if state["last_gather"] is not None:
                    dep(ld, state["last_gather"])
                state["reload"] = ld

            reload_tables()  # p0 = z (p_slice written during stage A)

            def iteration(it: int, last: bool):
                g_outs = {}

                def emit_gather(c):
                    g_out = pch.tile([P, GCH], F32, name=f"g_out{c % 2}",
                                     bufs=1)
                    gather = nc.gpsimd.ap_gather(
                        out_ap=g_out[:].rearrange("p (n o) -> p n o", o=1),
                        in_ap=table[:].rearrange("p (n o) -> p n o", o=1),
                        idxs_ap=eidx_sb[:, c * (GCH // 16):
                                        (c + 1) * (GCH // 16)],
                        channels=P, num_elems=NE, d=1, num_idxs=GCH)
                    pool_chain(gather)
                    if state["reload"] is not None:
                        dep(gather, state["reload"])
                        if c == n_gch - 1:
                            state["reload"] = None
                    if state["idx_loads"]:
                        for ld in state["idx_loads"]:
                            dep(gather, ld)
                        state["idx_loads"] = []
                    if state["gout_ring"][c % 2] is not None:
                        dep(gather, state["gout_ring"][c % 2])
                    state["last_gather"] = gather
                    # weights for this chunk (single buffer)
                    wld = nc.sync.dma_start(out=wch[:], in_=ew_e[c])
                    if state["wch_last_reader"] is not None:
                        dep(wld, state["wch_last_reader"])
                    mult = nc.vector.tensor_tensor(out=g_out[:], in0=g_out[:],
                                                   in1=wch[:], op=ALU.mult)
                    dep(mult, gather)
                    dep(mult, wld)
                    state["wch_last_reader"] = mult
                    # z chunk for these subs
                    s0 = c * SPG
                    s1 = min(s0 + SPG, n_subs)
                    z0 = s0 * D_SUB
                    z1 = min(R, s1 * D_SUB)
                    zch = pch.tile([LAB, SPG * D_SUB], F32,
                                   name=f"zch{c % 2}", bufs=1)
                    zld = nc.sync.dma_start(out=zch[:, 0:z1 - z0],
                                            in_=z_dram[:, z0:z1])
                    g_outs[c] = (g_out, mult, zch, zld)

                def emit_sub(sub):
                    c, sl = sub // SPG, sub % SPG
                    g_out, mult, zch, zld = g_outs[c]
                    sl0 = sl * CH_SUB
                    seg = g_out[:, sl0:sl0 + CH_SUB]
                    scan = nc.vector.tensor_tensor_scan(
                        out=seg, data0=ones[:].to_broadcast([P, CH_SUB]),
                        data1=seg, initial=0.0,
                        op0=ALU.mult, op1=ALU.add)
                    dep(scan, mult)
                    aw = aw_t[sub % 2]
                    ext = nc.gpsimd.ap_gather(
                        out_ap=aw[:].rearrange("p (n o) -> p n o", o=1),
                        in_ap=seg.rearrange("p (n o) -> p n o", o=1),
                        idxs_ap=xidx_sb[:, sub * (D_SUB // 16):
                                        (sub + 1) * (D_SUB // 16)],
                        channels=P, num_elems=CH_SUB, d=1, num_idxs=D_SUB)
                    pool_chain(ext)
                    dep(ext, scan)
                    if state["aw_ring"][sub % 2] is not None:
                        dep(ext, state["aw_ring"][sub % 2])
                    psc = ps.tile([LAB, D_SUB], F32, name="psc", space="PSUM")
                    mm1 = nc.tensor.matmul(out=psc[:], lhsT=hmask[:],
                                           rhs=aw[:], start=True, stop=False)
                    dep(mm1, ext)
                    mm2 = nc.tensor.matmul(out=psc[:, 1:D_SUB],
                                           lhsT=hmaskn[:],
                                           rhs=aw[:, 0:D_SUB - 1],
                                           start=False, stop=True)
                    state["aw_ring"][sub % 2] = mm2
                    # out = psc + alpha * z   -> sbuf, then DMA to p_slice
                    pt = pch.tile([LAB, D_SUB], F32, name=f"pt{sub % 2}",
                                  bufs=1)
                    d0 = sub * D_SUB
                    d1 = min(d0 + D_SUB, R)
                    stt = nc.vector.scalar_tensor_tensor(
                        out=pt[:], in0=zch[:, sl * D_SUB:(sl + 1) * D_SUB],
                        scalar=cfg.ALPHA, in1=psc[:],
                        op0=ALU.mult, op1=ALU.add)
                    dep(stt, mm2)
                    dep(stt, zld)
                    for w in state["ps_writers"]:
                        if w is not None:
                            dep(stt, w)
                    wr = nc.sync.dma_start(out=p_slice[:, d0:d1],
                                           in_=pt[:, 0:d1 - d0])
                    return wr

                writers = []
                for c in range(n_gch):
                    emit_gather(c)
                    for sl in range(SPG):
                        sub = c * SPG + sl
                        if sub < n_subs:
                            writers.append(emit_sub(sub))
                state["ps_writers"] = writers
                if not last:
                    reload_tables()

            for it in range(cfg.ITERS):
                iteration(it, last=(it == cfg.ITERS - 1))

            # ------------- epilogue: transpose + log_softmax -------------
            # p_slice holds final p [16, R]; process 4 chunks of 128 nodes
            # at a time.
            total_chunks = (R + 127) // 128
            b = 0
            done = 0
            last_stt = state["ps_writers"]
            while done < total_chunks:
                nchk = min(4, total_chunks - done)
                n0 = done * 128
                n1 = min(n0 + 4 * 128, R)
                pin = pch.tile([LAB, 4 * 128], F32, name="pin")
                pld = nc.sync.dma_start(out=pin[:, 0:n1 - n0],
                                        in_=p_slice[:, n0:n1])
                for w in last_stt:
                    dep(pld, w)
                last_stt = []
                ps3 = ps.tile([P, 4 * LAB], F32, name="ps3", space="PSUM")
                tr = []
                for t in range(nchk):
                    trr = nc.tensor.transpose(
                        out=ps3[:, t * LAB:(t + 1) * LAB],
                        in_=pin[:, t * 128:(t + 1) * 128],
                        identity=ident[0:LAB, :])
                    dep(trr, pld)
                    tr.append(trr)
                sb = pch.tile([P, 4, LAB], F32, name="sm_sb")
                cp = nc.vector.tensor_copy(
                    out=sb[:, 0:nchk, :].rearrange("p a l -> p (a l)"),
                    in_=ps3[:, 0:nchk * LAB])
                for trr in tr:
                    dep(cp, trr)
                mx = pch.tile([P, 4, 1], F32, name="sm_mx")
                nc.vector.tensor_reduce(out=mx[:, 0:nchk], in_=sb[:, 0:nchk],
                                        axis=mybir.AxisListType.X, op=ALU.max)
                nc.vector.tensor_tensor(
                    out=sb[:, 0:nchk], in0=sb[:, 0:nchk],
                    in1=mx[:, 0:nchk].to_broadcast([P, nchk, LAB]),
                    op=ALU.subtract)
                ex = pch.tile([P, 4, LAB], F32, name="sm_ex")
                nc.scalar.activation(out=ex[:, 0:nchk], in_=sb[:, 0:nchk],
                                     func=AF.Exp)
                sm = pch.tile([P, 4, 1], F32, name="sm_sm")
                nc.vector.tensor_reduce(out=sm[:, 0:nchk], in_=ex[:, 0:nchk],
                                        axis=mybir.AxisListType.X, op=ALU.add)
                lg = pch.tile([P, 4, 1], F32, name="sm_lg")
                nc.scalar.activation(out=lg[:, 0:nchk], in_=sm[:, 0:nchk],
                                     func=AF.Ln)
                nc.vector.tensor_tensor(
                    out=sb[:, 0:nchk], in0=sb[:, 0:nchk],
                    in1=lg[:, 0:nchk].to_broadcast([P, nchk, LAB]),
                    op=ALU.subtract)
                nc.sync.dma_start(
                    out=y_e[:].rearrange("(x p) l -> p x l", p=P)[
                        :, 4 * b:4 * b + nchk, :],
                    in_=sb[:, 0:nchk, :])
                done += nchk
                b += 1
            for _f in reversed(_frees):
                _f()
    nc.compile()
    return nc


def unpack_output(results, cfg: Cfg):
    out = np.zeros((cfg.N, cfg.LAB), np.float32)
    for k in range(cfg.NCS):
        y = results[k]["y"]
        out[k * cfg.R:(k + 1) * cfg.R] = y[0:cfg.R]
    return out


# ---------------------------------------------------------------------------
_CACHE = {}


def kernel(**inputs):
    import numpy as np
    from concourse.bass_utils import run_bass_kernel_spmd

    cfg = Cfg()
    in_maps, _meta = prep_host(inputs, cfg)
    key = (cfg.CH_SUB, cfg.n_subs)
    if key not in _CACHE:
        _CACHE[key] = build_kernel(cfg)
    nc = _CACHE[key]
    r = run_bass_kernel_spmd(nc, in_maps, list(range(cfg.NCS)))
    return unpack_output(r.results, cfg)
